# revision 30
# baseline (speedup 1.0000x reference)
"""Trainium2 Bass kernel for nn_AttenuationToRainRate (dense_mlp).

Reference computation per (sample b, position t):
  style MLP: metadata (16) -> 64 -> 128 -> 64, split into 4 x (scale, bias)[8]
  main chain: x -> [w1] -> adain/lrelu -> [w2] -> adain/lrelu -> [w3] ->
              adain/lrelu -> [w4] -> adain/lrelu -> [w5] -> lrelu
  adain(h) = scale * (h - mean_c h) / (std_ddof1(h) + 1e-6) + bias

Design v8 (data-parallel over 8 cores, 32 samples each; 254 us vs the
347 us v3 baseline):
  Layout: [128, N] tiles, partition p = 8*s' + c (16 samples x 8 ch),
  free dim = positions, processed in 8 iters of (g in 2) x (k in 4) with
  2048 positions each.  Mean-removal folded into weights host-side
  (W' = W (I - J/8), b' = b - mean b).  fp16 activations everywhere;
  f32 only inside PSUM.

  L1 runs entirely off the PE: d1 = w1'*x + b1' is a per-partition
  affine of an x broadcast (DVE tensor_scalar), and var_1(x) =
  A x^2 + B x + C with host scalars A,B,C evaluated on a PACKED
  [64, 512] x view (p = 16*tau + s') so Square+AbsRecipSqrt run at
  FD 512 instead of the 2048-wide broadcast form.

  L2-4, per (layer, 1024-position half) chain: rank-1 bias matmuls
  (start) + block-diag W' matmuls (stop) -> d+b' in PSUM; ACT
  Square(d) -> dsq fp16; PE packed var matmul into [32, 512]
  (2 zero-padded [128,32] stationaries, one accumulation group);
  ACT AbsRecipSqrt -> r packed fp16; DMA-broadcast r to [128, 1024]
  (replicating access pattern [[512,16],[0,8],[1,512]], one DMA per
  512-chunk, issue alternating sync/gpsimd); custom DVE op
  ADAIN_APPLY_ANT: a = lrelu(s0 * d * r + s1) straight from PSUM
  (s0/s1 per-partition style scale/bias, alpha immediate) -> fp16 a.

  Software pipelining: iterations processed in pairs with the two
  iters' (layer, half) chains interleaved, and the next pair's L1
  emitted ahead so the PE always has runnable matmuls; PSUM = 3
  [128,1024] d-tiles + 2 var/L5 banks = 8 banks.

  L5: banded stationaries accumulate a packed [64, 512] PSUM tile
  (partition 16*tau + s'), out = Prelu(h5 + b5), DMA'd to a strided
  DRAM view so the gather is a pure reshape.

  All ACT functions (Square/AbsRecipSqrt/Prelu/Relu/Identity) live in
  one activation-table set, pinned via a Bacc subclass.
"""

import numpy as np

B_FULL, T = 256, 8192
NCORES = 8
BS = B_FULL // NCORES  # 32 samples per core
F = 16

CFG = {
    "sq_dve": 0,   # how many of the 6 per-iter Square half-ops run on DVE
}

_CACHE = {}


def _reset():
    _CACHE.clear()


# ------------------------------------------------------- custom DVE op

def _register_adain_op():
    from concourse import dve_ops
    from concourse.dve_spec import Spec, Src0, Src1, C0, C1, C2, maxx, lower
    from concourse.dve_uop import DveOpSpec

    name = "ADAIN_APPLY_ANT"
    if name in dve_ops._SUB_OPCODE_FOR_NAME:
        return next(o for o in dve_ops.OPS if o.name == name)

    # a = lrelu(e * r * s0 + s1); s0 = style scale, s1 = style bias
    _z = Src0 * Src1 * C0 + C1
    _body = maxx(_z, _z * C2)

    def _ref(in0, in1, s0, s1, imm2):
        z = in0.astype(np.float32) * in1 * s0 + s1
        return np.maximum(z, z * imm2)

    spec = Spec(body=_body, reference=_ref)
    shas = {}
    for ver in ("v3", "v4"):
        s = DveOpSpec(name=name, opcode=1, uops=lower(spec, ver=ver),
                      rd1_en=True)
        shas[ver] = s.sha(ver)
    op = dve_ops.DveOp(name, spec, subdim=False, uops_sha=shas)
    dve_ops.OPS.append(op)
    dve_ops._SUB_OPCODE_FOR_NAME[name] = (
        dve_ops._CUSTOM_DVE_ROW_BASE + len(dve_ops.OPS) - 1)
    dve_ops.CUSTOM_DVE_SPECS[name] = spec
    return op


# ----------------------------------------------------------------- host side

def _host_weights(inp):
    """Weight-derived constants in device layouts."""
    f64 = np.float64
    I8 = np.eye(8, dtype=f64)
    C = I8 - np.full((8, 8), 1.0 / 8.0, dtype=f64)  # output-centering

    w = {}
    # L1 (applied on DVE): centered per-channel vectors, tiled over samples
    w1 = np.asarray(inp["w1"], dtype=f64)            # (1, 8)
    b1 = np.asarray(inp["b1"], dtype=f64)            # (8,)
    w1p = (w1 @ C)[0]                                # (8,)
    b1p = b1 - b1.mean()
    w["w1c"] = np.tile(w1p, 16).reshape(128, 1)
    w["b1c"] = np.tile(b1p, 16).reshape(128, 1)
    # L1 variance quadratic: var_ddof1 = (A x^2 + B x + C)  (already /7)
    A = float((w1p ** 2).sum() / 7.0)
    B = float(2.0 * (w1p * b1p).sum() / 7.0)
    Cc = float((b1p ** 2).sum() / 7.0)
    w["_A"] = A
    w["_beta"] = B / (2.0 * A)
    w["_cprime"] = Cc - B * B / (4.0 * A) + 1e-12

    brow = np.zeros((1, 3 * 128), dtype=f64)
    for l in (2, 3, 4):
        W = np.asarray(inp[f"w{l}"], dtype=f64) @ C
        bp = np.asarray(inp[f"b{l}"], dtype=f64)
        bp = bp - bp.mean()
        wb = np.zeros((128, 128), dtype=f64)
        for s in range(16):
            wb[8 * s:8 * s + 8, 8 * s:8 * s + 8] = W
        w[f"wb{l}"] = wb
        brow[0, 128 * (l - 2):128 * (l - 1)] = np.tile(bp, 16)
    w["brow"] = brow
    w["onesr"] = np.ones((1, 512), dtype=f64)

    # packed-var stationaries (per half): vones[tp][8s+c, 16*tp+s] = 1
    for tp in range(2):
        m = np.zeros((128, 32), dtype=f64)
        for s in range(16):
            for c in range(8):
                m[8 * s + c, 16 * tp + s] = 1.0
        w[f"vones{tp}"] = m

    w5b = np.zeros((128, 4 * 64), dtype=f64)
    w5 = np.asarray(inp["w5"], dtype=f64)[:, 0]
    for tau in range(4):
        for s in range(16):
            for c in range(8):
                w5b[8 * s + c, 64 * tau + 16 * tau + s] = w5[c]
    w["w5b"] = w5b
    w["b5c"] = np.full((64, 1), float(np.asarray(inp["b5"], dtype=f64)[0]))

    w["mw1"] = np.asarray(inp["mw1"], dtype=f64)
    w["mw2"] = np.asarray(inp["mw2"], dtype=f64)
    w["mw3"] = np.asarray(inp["mw3"], dtype=f64)
    w["mb1c"] = np.asarray(inp["mb1"], dtype=f64).reshape(64, 1)
    w["mb2c"] = np.asarray(inp["mb2"], dtype=f64).reshape(128, 1)
    w["mb3c"] = np.asarray(inp["mb3"], dtype=f64).reshape(64, 1)

    out = {}
    for k, v in w.items():
        if k.startswith("_"):
            out[k] = v
            continue
        dt = np.float16 if k in _MM_STAT else np.float32
        out[k] = np.ascontiguousarray(np.asarray(v).astype(dt))
    return out


_WSHAPES = {
    "wb2": [128, 128], "wb3": [128, 128], "wb4": [128, 128],
    "brow": [1, 384], "onesr": [1, 512],
    "vones0": [128, 32], "vones1": [128, 32],
    "w5b": [128, 256], "b5c": [64, 1],
    "w1c": [128, 1], "b1c": [128, 1],
    "mw1": [16, 64], "mw2": [64, 128], "mw3": [128, 64],
    "mb1c": [64, 1], "mb2c": [128, 1], "mb3c": [64, 1],
}
# tensors that feed PE matmuls (fp16)
_MM_STAT = {"wb2", "wb3", "wb4", "w5b", "vones0", "vones1",
            "brow", "onesr"}


# --------------------------------------------------------------- device side

def build_program(cfg=None, consts=None):
    import concourse.bacc as bacc
    import concourse.mybir as mybir
    from concourse.ap import AP
    from concourse.tile import TileContext

    adain_op = _register_adain_op()

    cfg = dict(CFG if cfg is None else cfg)
    f32 = mybir.dt.float32
    f16 = mybir.dt.float16
    AF = mybir.ActivationFunctionType
    OP = mybir.AluOpType
    A_c, beta_c, cprime_c = consts["_A"], consts["_beta"], consts["_cprime"]

    class _KBacc(bacc.Bacc):
        _ACT_SET = "abs_reciprocal_sqrt_and_small"

        def insert_act_table_loads(self):
            import concourse.mybir as _mb
            from concourse.hw_specs import get_activation_tables
            has_activation = any(
                isinstance(i, _mb.InstActivation)
                for b in self.main_func.blocks
                for i in b.instructions
            )
            if not has_activation:
                return
            tables = []
            for name, funcs in get_activation_tables(self.m.arch).items():
                tables.append((name, funcs if name == self._ACT_SET else set()))
            bacc._bass_rust.insert_act_table_loads(self, tables)

    nc = _KBacc("TRN2", target_bir_lowering=False)
    x_d = nc.dram_tensor("x", [BS, T], f16, kind="ExternalInput")
    md_d = nc.dram_tensor("metadata", [BS, F], f32, kind="ExternalInput")
    y_d = nc.dram_tensor("y", [BS, T], f32, kind="ExternalOutput")
    wd = {name: nc.dram_tensor(name, shp, f16 if name in _MM_STAT else f32,
                               kind="ExternalInput")
          for name, shp in _WSHAPES.items()}

    XROW = T  # elements per x row in DRAM

    with TileContext(nc) as tc:
        with tc.tile_pool(name="const", bufs=1) as cp, \
             tc.tile_pool(name="scr", bufs=1, space="DRAM") as dp:

            # ---- constants to SBUF; style-MLP inputs first so the style
            # chain (which gates every fused op via scv/bcv) starts early
            cw = {}
            mdT = cp.tile([F, BS], f32, name="mdT")
            nc.sync.dma_start(out=mdT[:], in_=md_d.rearrange("s f -> f s"))
            _style_first = ["mw1", "mb1c", "mw2", "mb2c", "mw3", "mb3c"]
            _order = _style_first + [n for n in _WSHAPES if n not in _style_first]
            for name in _order:
                shp = _WSHAPES[name]
                t = cp.tile(shp, f16 if name in _MM_STAT else f32,
                            name=f"c_{name}")
                nc.sync.dma_start(out=t[:], in_=wd[name][:])
                cw[name] = t
            eps_s = cp.tile([64, 1], f32, name="eps_s")
            nc.vector.memset(eps_s[:], 1e-12)
            beta_s = cp.tile([64, 1], f32, name="beta_s")
            nc.vector.memset(beta_s[:], beta_c)
            cprime_s = cp.tile([64, 1], f32, name="cprime_s")
            nc.vector.memset(cprime_s[:], cprime_c)

            # ---- style MLP (per-core 32 samples)
            with tc.tile_pool(name="stp", bufs=1, space="PSUM") as sp:
                ps1 = sp.tile([64, BS], f32, name="ps1")
                nc.tensor.matmul(ps1[:], cw["mw1"][:], mdT[:],
                                 start=True, stop=True)
                s1 = cp.tile([64, BS], f32, name="s1")
                nc.scalar.activation(s1[:], ps1[:], AF.Relu, bias=cw["mb1c"][:])
                ps2 = sp.tile([128, BS], f32, name="ps2")
                nc.tensor.matmul(ps2[:], cw["mw2"][:], s1[:],
                                 start=True, stop=True)
                s2 = cp.tile([128, BS], f32, name="s2")
                nc.scalar.activation(s2[:], ps2[:], AF.Relu, bias=cw["mb2c"][:])
                ps3 = sp.tile([64, BS], f32, name="ps3")
                nc.tensor.matmul(ps3[:], cw["mw3"][:], s2[:],
                                 start=True, stop=True)
                sT = cp.tile([64, BS], f32, name="sT")
                nc.scalar.activation(sT[:], ps3[:], AF.Identity,
                                     bias=cw["mb3c"][:])

            # ---- per-(layer, supergroup) scale/bias vectors via DRAM trip
            # sT row = 16(l-1) + 2c + (0 scale / 1 bias), col = 16 sg + s'
            sT_d = dp.tile([64, BS], f32, name="sT_d")
            nc.gpsimd.dma_start(out=sT_d[:], in_=sT[:])
            scv = cp.tile([128, 8], f32, name="scv")   # scale, col j=(l-1)*2+sg
            bcv = cp.tile([128, 8], f32, name="bcv")   # bias
            for l in range(1, 5):
                for g in range(2):
                    j = (l - 1) * 2 + g
                    src_s = AP(tensor=sT_d[:].tensor,
                               offset=512 * (l - 1) + 16 * g,
                               ap=((1, 16), (64, 8)))
                    nc.gpsimd.dma_start(out=scv[:, j:j + 1], in_=src_s)
                    src_b = AP(tensor=sT_d[:].tensor,
                               offset=512 * (l - 1) + 32 + 16 * g,
                               ap=((1, 16), (64, 8)))
                    nc.gpsimd.dma_start(out=bcv[:, j:j + 1], in_=src_b)


            # ---------------- main loop
            with tc.tile_pool(name="pd", bufs=3, space="PSUM") as pdp, \
                 tc.tile_pool(name="pv", bufs=2, space="PSUM") as pvp, \
                 tc.tile_pool(name="xbc", bufs=2) as xbp, \
                 tc.tile_pool(name="xpk", bufs=5) as xkp, \
                 tc.tile_pool(name="d1p", bufs=5) as d1p, \
                 tc.tile_pool(name="dsqp", bufs=6) as dqp, \
                 tc.tile_pool(name="rpk", bufs=14) as rkp, \
                 tc.tile_pool(name="rbc", bufs=14) as rbp, \
                 tc.tile_pool(name="actp", bufs=12) as app, \
                 tc.tile_pool(name="outp", bufs=4) as opp:

                x_ap = x_d[:]

                bc_alt = [0]

                def bcast(dst_view, src_tile, src_row0, n_tau):
                    """dst[8s+c, 512*tp + t] = src[16*(row0+tp) + s, t]."""
                    sap = src_tile[:]
                    for tp in range(n_tau):
                        bsrc = AP(
                            tensor=sap.tensor,
                            offset=sap.offset + 16 * (src_row0 + tp) * 512,
                            ap=[[512, 16], [0, 8], [1, 512]])
                        eng = nc.gpsimd if bc_alt[0] & 1 else nc.sync
                        bc_alt[0] += 1
                        eng.dma_start(
                            out=dst_view[:, 512 * tp:512 * (tp + 1)],
                            in_=bsrc)

                xg_cache = {}

                def get_xg(g):
                    """Whole-g x broadcast [128, 8192] (one big DMA)."""
                    if g not in xg_cache:
                        xg = xbp.tile([128, 8192], f16, name=f"x_g{g}",
                                      tag="x_bc")
                        xsrc = AP(tensor=x_ap.tensor, offset=g * 16 * XROW,
                                  ap=[[XROW, 16], [0, 8], [1, 8192]])
                        nc.sync.dma_start(out=xg[:], in_=xsrc)
                        xg_cache[g] = xg
                    return xg_cache[g]

                def stage_l1(g, k):
                    """x DMAs + L1 chain (no PE); returns a1 tile."""
                    base = g * 16 * XROW + 2048 * k
                    x_bc = get_xg(g)[:, 2048 * k:2048 * (k + 1)]
                    # x packed [64, 512]: p=16tau+s <- x[.., 512tau+t]
                    x_pk = xkp.tile([64, 512], f16, name="x_pk", tag="x_pk")
                    xpsrc = AP(tensor=x_ap.tensor, offset=base,
                               ap=[[512, 4], [XROW, 16], [1, 512]])
                    nc.gpsimd.dma_start(out=x_pk[:], in_=xpsrc)

                    # L1 (no PE): d1 = sw1*x + sb1 ; r1 via quadratic var
                    d1 = d1p.tile([128, 2048], f16, name="d1", tag="d1")
                    nc.vector.tensor_scalar(
                        d1[:], x_bc, cw["w1c"][:], cw["b1c"][:],
                        OP.mult, OP.add)
                    u1 = rkp.tile([64, 512], f16, name="u1", tag="rpk")
                    nc.scalar.activation(u1[:], x_pk[:], AF.Square,
                                         bias=beta_s[:])
                    r1p = rkp.tile([64, 512], f16, name="r1p", tag="rpk")
                    nc.scalar.activation(r1p[:], u1[:],
                                         AF.Abs_reciprocal_sqrt,
                                         scale=A_c, bias=cprime_s[:])
                    r1bc = rbp.tile([128, 2048], f16, name="r1bc", tag="rbc")
                    bcast(r1bc[:, 0:1024], r1p, 0, 2)
                    bcast(r1bc[:, 1024:2048], r1p, 2, 2)
                    a1 = app.tile([128, 2048], f16, name="a1", tag="a")
                    for h in range(2):
                        sl = slice(1024 * h, 1024 * (h + 1))
                        nc.vector._custom_dve(
                            adain_op, out=a1[:, sl], in0=d1[:, sl],
                            in1=r1bc[:, sl], s0=scv[:, g:g + 1],
                            s1=bcv[:, g:g + 1], imm2=0.01)
                    return a1

                def chain_head(l, h, s_):
                    """d matmuls + square for one (layer, half) chain.
                    Returns context for chain_tail.  Keeping the var matmul
                    OUT of this part stops it from head-of-line-blocking the
                    next chain's (ready) d matmuls in the PE stream."""
                    g = s_["g"]
                    a_prev = s_["a"]
                    dt = pdp.tile([128, 1024], f32, name=f"d{l}{h}",
                                  tag="dt")
                    bsl = slice(128 * (l - 2), 128 * (l - 1))
                    # bias first (rank-1, start), then the two wb matmuls
                    # share one stationary (stop)
                    for tt in range(2):
                        psl = slice(512 * tt, 512 * (tt + 1))
                        nc.tensor.matmul(dt[:, psl], cw["brow"][:, bsl],
                                         cw["onesr"][:],
                                         start=True, stop=False)
                    for tt in range(2):
                        sl = slice(1024 * h + 512 * tt,
                                   1024 * h + 512 * (tt + 1))
                        psl = slice(512 * tt, 512 * (tt + 1))
                        nc.tensor.matmul(dt[:, psl], cw[f"wb{l}"][:],
                                         a_prev[:, sl],
                                         start=False, stop=True)
                    dsq = dqp.tile([128, 1024], f16, name=f"dsq{l}{h}",
                                   tag="dsq")
                    nc.scalar.activation(dsq[:], dt[:], AF.Square)
                    return (l, h, s_, dt, dsq)

                def chain_tail(ctx):
                    """var matmul + rsqrt + broadcast + fused apply."""
                    l, h, s_, dt, dsq = ctx
                    g = s_["g"]
                    j = (l - 1) * 2 + g
                    hs = slice(1024 * h, 1024 * (h + 1))
                    vb = pvp.tile([32, 512], f32, name=f"vb{l}{h}",
                                  tag="vb")
                    for tp in range(2):
                        nc.tensor.matmul(vb[:], cw[f"vones{tp}"][:],
                                         dsq[:, 512 * tp:512 * (tp + 1)],
                                         start=(tp == 0), stop=(tp == 1))
                    rp = rkp.tile([32, 512], f16, name=f"rp{l}{h}",
                                  tag="rpk")
                    nc.scalar.activation(rp[:], vb[:],
                                         AF.Abs_reciprocal_sqrt,
                                         scale=1.0 / 7.0,
                                         bias=eps_s[0:32, :])
                    rb = rbp.tile([128, 1024], f16, name=f"rb{l}{h}",
                                  tag="rbc")
                    bcast(rb[:], rp, 0, 2)
                    nc.vector._custom_dve(
                        adain_op, out=s_["anew"][:, hs], in0=dt[:],
                        in1=rb[:], s0=scv[:, j:j + 1], s1=bcv[:, j:j + 1],
                        imm2=0.01)

                def stage_l5(s_):
                    g, k, a_prev = s_["g"], s_["k"], s_["a"]
                    h5 = pvp.tile([64, 512], f32, name="h5", tag="vb")
                    for tau in range(4):
                        sl = slice(512 * tau, 512 * (tau + 1))
                        nc.tensor.matmul(
                            h5[:], cw["w5b"][:, 64 * tau:64 * (tau + 1)],
                            a_prev[:, sl],
                            start=(tau == 0), stop=(tau == 3))
                    oc = opp.tile([64, 512], f32, name="oc", tag="oc")
                    nc.scalar.activation(oc[:], h5[:], AF.Prelu,
                                         bias=cw["b5c"][:], alpha=0.01)
                    ydst = y_d.rearrange(
                        "(sg sp) (kk tau n) -> sg kk tau sp n",
                        sg=2, kk=4, tau=4, n=512)[g, k]
                    # oc partition-major order (p = 16 tau + sp) matches
                    # the (tau, sp, n) iteration of ydst
                    nc.sync.dma_start(out=ydst, in_=oc[:])

                # software pipeline: iterations in GROUPS of 3, the group's
                # 6 (layer, half) chains emitted as a depth-3 sliding window
                # (chain k's tail lands just before chain k+3's head, which
                # reuses its PSUM buffer), and the NEXT group's L1 emitted
                # before the current group's L2-5
                iters = [(g, k) for g in range(2) for k in range(4)]
                groups = [iters[i:i + 4] for i in range(0, len(iters), 4)]
                WIN = 3  # == pd bufs

                def l1_group(gr):
                    return [{"g": g, "k": k, "a": stage_l1(g, k)}
                            for (g, k) in gr]

                st_pending = l1_group(groups[0])
                for pi, gr in enumerate(groups):
                    st = st_pending
                    if pi + 1 < len(groups):
                        st_next = l1_group(groups[pi + 1])
                    for l in range(2, 5):
                        for s_ in st:
                            s_["anew"] = app.tile([128, 2048], f16,
                                                  name=f"a{l}", tag="a")
                        chains = [(h, s_) for h in range(2) for s_ in st]
                        pending = []
                        for h, s_ in chains:
                            if len(pending) >= WIN:
                                chain_tail(pending.pop(0))
                            pending.append(chain_head(l, h, s_))
                        for ctx in pending:
                            chain_tail(ctx)
                        for s_ in st:
                            s_["a"] = s_["anew"]
                    for s_ in st:
                        stage_l5(s_)
                    if pi + 1 < len(groups):
                        st_pending = st_next

    nc.compile()
    return nc


# ------------------------------------------------------------------- runner

def _get_program_and_weights(inputs):
    wts = _host_weights(inputs)
    key = tuple(sorted(CFG.items()))
    if key not in _CACHE:
        _CACHE[key] = build_program(CFG, consts=wts)
    return _CACHE[key], wts


def _make_in_maps(inputs, wts):
    x = np.ascontiguousarray(
        np.asarray(inputs["x"], dtype=np.float32).reshape(B_FULL, T).astype(
            np.float16))
    md = np.ascontiguousarray(np.asarray(inputs["metadata"], dtype=np.float32))
    dev = {k: v for k, v in wts.items() if not k.startswith("_")}
    in_maps = []
    for i in range(NCORES):
        m = dict(dev)
        m["x"] = np.ascontiguousarray(x[BS * i:BS * (i + 1)])
        m["metadata"] = np.ascontiguousarray(md[BS * i:BS * (i + 1)])
        in_maps.append(m)
    return in_maps


def run_spmd(inputs, trace=False):
    """Run on all 8 cores; returns (y_full, BassKernelResults)."""
    from concourse.bass_utils import run_bass_kernel_spmd
    nc, wts = _get_program_and_weights(inputs)
    in_maps = _make_in_maps(inputs, wts)
    res = run_bass_kernel_spmd(nc, in_maps, core_ids=list(range(NCORES)),
                               trace=trace)
    y = np.concatenate([np.asarray(r["y"]) for r in res.results], axis=0)
    y = y.reshape(B_FULL, 1, T).astype(np.float32)
    return y, res


def kernel(**inputs):
    y, _ = run_spmd(inputs, trace=False)
    return y
